# revision 29
# baseline (speedup 1.0000x reference)
"""KoLeo loss (distributed) on 8 Trainium2 NeuronCores.

Strategy: data-parallel over rows, fp8 DoubleRow GEMM. Host normalizes x
(fp64), scales by 16, quantizes to fp8e4 (e4m3), transposes to [D, B] and
stages it per-core ROTATED so each core's own 1024 rows sit at columns
0..1024 — the same program slice is the stationary operand on every core
(no separate lhsT input). Each core computes its [1024, 8192] slice of the
scaled Gram matrix with DoubleRow fp8 matmuls (2 k-chunks per instruction,
0.5 PE cycles/row = 4x bf16), streaming quarter-row [128, 4, 512] tiles
through the 8 PSUM banks.

Top-8 extraction works around two hardware limits (DVE allows only ONE
PSUM operand per instruction; GpSimd cannot touch PSUM or run
tensor ops in this walrus build): the ACT engine drains most PSUM
quarters to SBUF bf16 copies (scalar.copy), and the DVE folds those
copies into a per-row-tile running max with 2x-mode tensor_tensor(max)
— occasionally consuming a quarter directly from PSUM paired against a
banked copy ("gulp") where that balances the engines. Two more fold
levels and the DVE max8 instruction reduce each row-tile to its top-8
slot maxima. Row-tiles 0-3 run column-round-major so the working set
tracks DMA slab arrival (single queue, strict column order); rows 4-7
run row-major. Because rows are unit-norm, the self-dot (=256 scaled)
always ranks first; nearest-neighbor distances follow from
d^2 = 2 - 2*dot. Host reduces the 8x[8,128,8] top-8 tables to the
scalar loss in float64. Fold-slot collisions (two of a row's top-2
neighbors, or self and a neighbor, landing in the same max-slot)
affect ~0.1% of rows and perturb the loss by ~1e-4 relative; fp8 input
quantization contributes ~2e-3 — both far under the 2e-2 gate.
Engine budget per core (TimelineSim): PE 60us, ACT 57us, DVE 42us,
DMA 24us -> 81.4us total vs 233.7us for the bf16 max8-only baseline.
"""

import sys

sys.path.insert(0, "/opt/trn_rl_repo")

import numpy as np
import ml_dtypes

import concourse.bass as bass
import concourse.tile as tile
from concourse import mybir
from concourse.alu_op_type import AluOpType
from concourse.bass import ds, ts
from concourse.vector_clock import ScopedClock
from concourse.bass_utils import run_bass_kernel_spmd

B = 8192
D = 1024
NCORES = 8
P = 128
MT = (B // NCORES) // P  # 8 row-tiles per core
KC = D // P  # 8 k-chunks of 128
KP = KC // 2  # 4 DoubleRow steps (256-contraction each)
NG = 4  # psum groups per row-tile (each 4 banks = 2048 cols)
GW = 2048  # columns per group
SCALE = 16.0

ST_BUFS = 6
CP_BUFS = 12
DEFER_CASC = ()
DEFER_Q = 1
PLAN1_STR = "FFFF"
PLAN2_STR = "FFFG"

TOPK = 2
GATE_THRESHOLD = 0.5
GATE_ALPHA = 0.1
EPS = 1e-8


class PatchedTileContext(tile.TileContext):
    """The tail drain in this walrus build only tolerates a single sem wait
    per instruction; spill the rest onto standalone wait instructions."""

    def _drain_and_barrier(self, tick_clock, wait_clock):
        nc = self.nc
        drain_inst = nc.sync.drain()
        wait_clock.add_sem_waits(
            drain_inst.ins, ScopedClock({None: tick_clock.global_clock})
        )
        si = drain_inst.ins.sync_info
        if si is not None and len(si.on_wait) > 1:
            waits = list(si.on_wait)
            si.on_wait = waits[:1]
            id2sem = {h.num: h for h in self.sems.allocated().values()}
            for w in waits[1:]:
                nc.sync.wait_ge(id2sem[w.id], w.wait_value)
        nc.all_engine_barrier()
        popped = nc._tile_sem_poison_stack.pop()
        assert popped is self._sem_poison
        nc.clear_and_free_semaphores(list(self.sems.allocated().values()))
        nc.all_engine_barrier()


def _split_excess_waits(nc, max_waits=1):
    """This walrus build rejects instructions carrying more than one sem
    wait; hoist extras onto standalone EventSemaphore instructions placed
    immediately before the over-subscribed instruction on the same engine
    (engines dispatch in order, so this is semantically identical)."""
    for fn in nc.m.functions:
        for bb in fn.blocks:
            insts = bb.instructions
            out = []
            for inst in insts:
                si = inst.sync_info
                if si is not None and len(si.on_wait) > max_waits:
                    waits = list(si.on_wait)
                    for w in waits[:-max_waits]:
                        ev = mybir.InstEventSemaphore(
                            name=nc.get_next_instruction_name(), ins=[], outs=[]
                        )
                        ev.engine = inst.engine
                        ev.sync_info = mybir.SyncInfo(on_wait=[w], on_update=[])
                        out.append(ev)
                    si.on_wait = waits[-max_waits:]
                out.append(inst)
            insts[:] = out


def build_program():
    nc = bass.Bass()
    xq_d = nc.declare_dram_parameter(
        "xq", [P, KC, B], mybir.dt.float8e4, isOutput=False
    )
    out_d = nc.declare_dram_parameter(
        "top8", [MT, P, 8], mybir.dt.float32, isOutput=True
    )

    with PatchedTileContext(nc) as tc:
        with (
            tc.tile_pool(name="xq_pool", bufs=NCORES) as xq_pool,
            tc.tile_pool(name="st_pool", bufs=ST_BUFS) as st_pool,
            tc.tile_pool(name="cp_pool", bufs=CP_BUFS) as cp_pool,
            tc.tile_pool(name="stg_pool", bufs=3) as stg_pool,
            tc.tile_pool(name="acc_pool", bufs=1) as acc_pool,
            tc.tile_pool(name="psum", bufs=2, space=bass.MemorySpace.PSUM) as psum_pool,
        ):
            # resident fp8 [128, KC, B]; one tile per 1024-column slab so
            # matmuls only depend on the slab they read
            xq_sb = [
                xq_pool.tile([P, KC, 1024], mybir.dt.float8e4, name="xq_rez")
                for _ in range(NCORES)
            ]
            # single queue in strict column order so slab k lands at ~2.9k us
            # (two queues interleave on the DMA bus and scramble arrival);
            # slabs 0-1 go in halves so the first fills start sooner
            for s in range(2):
                for h in range(2):
                    nc.sync.dma_start(
                        xq_sb[s][:, :, ds(h * 512, 512)],
                        xq_d[:, :, ds(s * 1024 + h * 512, 512)],
                    )
            for s in range(2, NCORES):
                nc.sync.dma_start(xq_sb[s][:], xq_d[:, :, ds(s * 1024, 1024)])

            # warm up the PE HAM clock gate during the DMA prologue so the
            # real matmuls run at full clock from the start
            warm_sb = acc_pool.tile([P, 512], mybir.dt.float8e4)
            nc.gpsimd.memset(warm_sb[:], 0.0)
            warm_ps = psum_pool.tile([P, 4, 512], mybir.dt.float32, name="psum")
            for i in range(12):
                nc.tensor.matmul(warm_ps[:, i % 4], warm_sb[:, :P], warm_sb[:])

            l2 = acc_pool.tile([P, 4, 512], mybir.dt.bfloat16)
            l3 = acc_pool.tile([P, 2, 512], mybir.dt.bfloat16)
            l4 = acc_pool.tile([P, 512], mybir.dt.bfloat16)
            out_sb = acc_pool.tile([P, MT, 8], mybir.dt.float32)

            def rhs_ap(kp, col0, width):
                """[128, 2, width] fp8 slice covering k-chunks 2kp,2kp+1."""
                s = col0 // 1024
                o = col0 % 1024
                return xq_sb[s][:, ds(2 * kp, 2), ds(o, width)]

            def fill(pst, m, q):
                """4 DoubleRow accumulation chains -> quarter-row [128,4,512]."""
                for j in range(4):
                    col0 = q * 2048 + j * 512
                    for kp in range(KP):
                        nc.tensor.matmul(
                            pst[:, j],
                            rhs_ap(kp, m * P, P),
                            rhs_ap(kp, col0, 512),
                            start=(kp == 0),
                            stop=(kp == KP - 1),
                            perf_mode=mybir.MatmulPerfMode.DoubleRow,
                        )

            sts = {}
            cps = {}
            stages = {}

            def do_cp(m, q, ps):
                c = cp_pool.tile([P, 4, 512], mybir.dt.bfloat16, name="cp")
                nc.scalar.copy(c[:], ps[:])
                cps[(m, q)] = c

            def st_of(m):
                if m not in sts:
                    sts[m] = st_pool.tile([P, 4, 512], mybir.dt.bfloat16, name="st")
                return sts[m]

            def merge_cp(m, q):
                # fold a banked copy into the rt's running max (bf16 2x mode);
                # first merge pairs the first two copies
                st = st_of(m)
                a = cps.pop((m, q))
                b = cps.pop((m, q - 1), None)
                if b is not None:
                    nc.vector.tensor_tensor(st[:], a[:], b[:], AluOpType.max)
                else:
                    nc.vector.tensor_tensor(st[:], a[:], st[:], AluOpType.max)

            def do_gulp(m, ps, against_cp=None, split=False):
                # one PSUM operand per DVE instruction; the second operand is
                # either a banked copy (st not started) or the running max
                st = st_of(m)
                if split:
                    # consume in 2-bank halves so the tail chain after the
                    # final matmul is half as long
                    for h in range(2):
                        nc.vector.tensor_tensor(
                            st[:, ds(2 * h, 2)],
                            ps[:, ds(2 * h, 2)],
                            st[:, ds(2 * h, 2)],
                            AluOpType.max,
                        )
                    return
                other = cps.pop((m, against_cp))[:] if against_cp is not None else st[:]
                nc.vector.tensor_tensor(st[:], ps[:], other, AluOpType.max)

            def cascade(m):
                if m in stages:
                    # M-plan: merge the four exact per-quarter top-8 tables
                    stg = stages.pop(m)
                    nc.vector.max(out_sb[:, m], stg[:].rearrange("p a b -> p (a b)"))
                    nc.sync.dma_start(out_d[m], out_sb[:, m])
                    return
                st = sts.pop(m)
                nc.vector.tensor_tensor(
                    l3[:], st[:, ds(0, 2)], st[:, ds(2, 2)], AluOpType.max
                )
                nc.vector.tensor_tensor(l4[:], l3[:, 0], l3[:, 1], AluOpType.max)
                nc.vector.max(out_sb[:, m], l4[:])
                nc.sync.dma_start(out_d[m], out_sb[:, m])

            # Per-rt consumer plans (DVE/ACT us per rt):
            #  A: gulp q1 against cp0, merge cp2/cp3 later (DVE 6.7, ACT 6.3)
            #  B: copies first, single gulp at q3 vs running max (6.7, 6.3)
            #  F: all four quarters copied, three bf16 merges (5.5, 8.4)
            def consume(m, q, ps, plan):
                if plan == "A":
                    if q == 0:
                        do_cp(m, q, ps)
                    elif q == 1:
                        do_gulp(m, ps, against_cp=0)
                    else:
                        do_cp(m, q, ps)
                        merge_cp(m, q)
                elif plan in ("B", "S"):
                    if q < 2:
                        do_cp(m, q, ps)
                        if q == 1:
                            merge_cp(m, q)
                    elif q == 2:
                        do_cp(m, q, ps)
                        merge_cp(m, q)
                    else:
                        do_gulp(m, ps, split=(plan == "S" or m == MT - 1))
                elif plan == "C":
                    # ACT-light chain: one copy, then in-place PSUM gulps
                    if q == 0:
                        do_cp(m, q, ps)
                    elif q == 1:
                        do_gulp(m, ps, against_cp=0)
                    else:
                        do_gulp(m, ps, split=(q == 3 and m == MT - 1))
                elif plan == "M":
                    # no ACT at all: exact per-quarter top-8 straight from
                    # PSUM on the DVE (soaks up its idle head window)
                    if m not in stages:
                        stages[m] = stg_pool.tile([P, 4, 8], mybir.dt.float32, name="stg")
                    nc.vector.max(stages[m][:, q], ps[:].rearrange("p a b -> p (a b)"))
                elif plan == "G":
                    # ACT-light endgame plan: only 2 copies; the last quarter
                    # is consumed by split gulps so the tail chain is short
                    if q == 0:
                        do_cp(m, q, ps)
                    elif q == 1:
                        do_gulp(m, ps, against_cp=0)
                    elif q == 2:
                        do_cp(m, q, ps)
                        merge_cp(m, q)
                    else:
                        do_gulp(m, ps, split=True)
                else:  # F
                    do_cp(m, q, ps)
                    if q >= 1:
                        merge_cp(m, q)
                if q == 3 and m not in DEFER_CASC:
                    cascade(m)

            # rts 0-3 column-round-major so the working set tracks DMA slab
            # arrival: round q touches only slabs 2q, 2q+1
            PLAN1 = {m: PLAN1_STR[m] for m in range(4)}
            for q in range(4):
                for m in range(4):
                    ps = psum_pool.tile([P, 4, 512], mybir.dt.float32, name="psum")
                    fill(ps, m, q)
                    consume(m, q, ps, PLAN1[m])

            # rts 4-7 row-major (all slabs resident by now); alternate the
            # ACT-heavy plan F with plan B to balance ACT and DVE
            PLAN2 = {m: PLAN2_STR[m - 4] for m in range(4, MT)}
            for m in range(4, MT):
                for q in range(4):
                    ps = psum_pool.tile([P, 4, 512], mybir.dt.float32, name="psum")
                    fill(ps, m, q)
                    consume(m, q, ps, PLAN2[m])
                    # deferred cascades run while the next rt's fills stream,
                    # keeping the final rt's tail chain unqueued
                    if m - 1 in DEFER_CASC and q == DEFER_Q:
                        cascade(m - 1)

    _split_excess_waits(nc)
    return nc


_nc_cache = None


def kernel(x: np.ndarray) -> np.ndarray:
    global _nc_cache
    assert x.shape == (B, D)

    # --- host: normalize (fp64), scale, quantize, transpose, rotate ---
    x64 = x.astype(np.float64)
    norm = np.sqrt(np.sum(x64 * x64, axis=1, keepdims=True))
    xn = x64 / np.maximum(norm, EPS)
    xq = (xn.T * SCALE).astype(ml_dtypes.float8_e4m3)  # [D, B]
    # [D, B] -> [KC, 128, B] -> [128, KC, B]
    xq = np.ascontiguousarray(xq.reshape(KC, P, B).transpose(1, 0, 2))

    in_maps = []
    for c in range(NCORES):
        r = c * (B // NCORES)
        rolled = np.concatenate((xq[:, :, r:], xq[:, :, :r]), axis=2)
        in_maps.append({"xq": np.ascontiguousarray(rolled)})

    if _nc_cache is None:
        _nc_cache = build_program()
    res = run_bass_kernel_spmd(_nc_cache, in_maps, list(range(NCORES)))

    # --- host: reduce top-8 tables to the scalar loss (fp64) ---
    # top8[c][mt, p, v] -> row c*1024 + mt*128 + p (rotation leaves each
    # core's own rows in place, so the row mapping matches the baseline)
    tops = np.stack([res.results[c]["top8"] for c in range(NCORES)])
    v = tops.reshape(B, 8).astype(np.float64) / (SCALE * SCALE)
    # rank 0 is the self-dot (~1.0); ranks 1..TOPK are the nearest neighbors
    vk = v[:, 1 : 1 + TOPK]  # [B, TOPK]
    d2 = np.maximum(2.0 - 2.0 * vk, 0.0)
    distances = np.sqrt(d2).reshape(-1)
    losses = -np.log(distances + EPS)
    alpha = max(GATE_ALPHA, 1e-6)
    gate = 1.0 / (1.0 + np.exp(-(losses - GATE_THRESHOLD) / alpha))
    lg = losses * gate
    weighted_mean = lg.mean()
    gated_mean = lg.sum() / max(gate.sum(), 1.0)
    out = 0.5 * weighted_mean + 0.5 * gated_mean
    return np.array(out, dtype=np.float32)


# revision 30
# speedup vs baseline: 1.0077x; 1.0077x over previous
"""KoLeo loss (distributed) on 8 Trainium2 NeuronCores.

Strategy: data-parallel over rows, fp8 DoubleRow GEMM. Host normalizes x
(fp64), scales by 16, quantizes to fp8e4 (e4m3), transposes to [D, B] and
stages it per-core ROTATED so each core's own 1024 rows sit at columns
0..1024 — the same program slice is the stationary operand on every core
(no separate lhsT input). Each core computes its [1024, 8192] slice of the
scaled Gram matrix with DoubleRow fp8 matmuls (2 k-chunks per instruction,
0.5 PE cycles/row = 4x bf16), streaming quarter-row [128, 4, 512] tiles
through the 8 PSUM banks.

Top-8 extraction works around two hardware limits (DVE allows only ONE
PSUM operand per instruction; GpSimd cannot touch PSUM or run
tensor ops in this walrus build): the ACT engine drains most PSUM
quarters to SBUF bf16 copies (scalar.copy), and the DVE folds those
copies into a per-row-tile running max with 2x-mode tensor_tensor(max)
— occasionally consuming a quarter directly from PSUM paired against a
banked copy ("gulp") where that balances the engines. Two more fold
levels and the DVE max8 instruction reduce each row-tile to its top-8
slot maxima. Row-tiles 0-3 run column-round-major so the working set
tracks DMA slab arrival (single queue, strict column order); rows 4-7
run row-major. Because rows are unit-norm, the self-dot (=256 scaled)
always ranks first; nearest-neighbor distances follow from
d^2 = 2 - 2*dot. Host reduces the 8x[8,128,8] top-8 tables to the
scalar loss in float64. Fold-slot collisions (two of a row's top-2
neighbors, or self and a neighbor, landing in the same max-slot)
affect ~0.1% of rows and perturb the loss by ~1e-4 relative; fp8 input
quantization contributes ~2e-3 — both far under the 2e-2 gate.
Engine budget per core (TimelineSim): PE 60us, ACT 57us, DVE 42us,
DMA 24us -> 81.4us total vs 233.7us for the bf16 max8-only baseline.
"""

import sys

sys.path.insert(0, "/opt/trn_rl_repo")

import numpy as np
import ml_dtypes

import concourse.bass as bass
import concourse.tile as tile
from concourse import mybir
from concourse.alu_op_type import AluOpType
from concourse.bass import ds, ts
from concourse.vector_clock import ScopedClock
from concourse.bass_utils import run_bass_kernel_spmd

B = 8192
D = 1024
NCORES = 8
P = 128
MT = (B // NCORES) // P  # 8 row-tiles per core
KC = D // P  # 8 k-chunks of 128
KP = KC // 2  # 4 DoubleRow steps (256-contraction each)
NG = 4  # psum groups per row-tile (each 4 banks = 2048 cols)
GW = 2048  # columns per group
SCALE = 16.0

ST_BUFS = 6
CP_BUFS = 28
DEFER_CASC = ()
DEFER_Q = 1
PLAN1_STR = "FFFF"
PLAN2_STR = "FFFG"

TOPK = 2
GATE_THRESHOLD = 0.5
GATE_ALPHA = 0.1
EPS = 1e-8


class PatchedTileContext(tile.TileContext):
    """The tail drain in this walrus build only tolerates a single sem wait
    per instruction; spill the rest onto standalone wait instructions."""

    def _drain_and_barrier(self, tick_clock, wait_clock):
        nc = self.nc
        drain_inst = nc.sync.drain()
        wait_clock.add_sem_waits(
            drain_inst.ins, ScopedClock({None: tick_clock.global_clock})
        )
        si = drain_inst.ins.sync_info
        if si is not None and len(si.on_wait) > 1:
            waits = list(si.on_wait)
            si.on_wait = waits[:1]
            id2sem = {h.num: h for h in self.sems.allocated().values()}
            for w in waits[1:]:
                nc.sync.wait_ge(id2sem[w.id], w.wait_value)
        nc.all_engine_barrier()
        popped = nc._tile_sem_poison_stack.pop()
        assert popped is self._sem_poison
        nc.clear_and_free_semaphores(list(self.sems.allocated().values()))
        nc.all_engine_barrier()


def _split_excess_waits(nc, max_waits=1):
    """This walrus build rejects instructions carrying more than one sem
    wait; hoist extras onto standalone EventSemaphore instructions placed
    immediately before the over-subscribed instruction on the same engine
    (engines dispatch in order, so this is semantically identical)."""
    for fn in nc.m.functions:
        for bb in fn.blocks:
            insts = bb.instructions
            out = []
            for inst in insts:
                si = inst.sync_info
                if si is not None and len(si.on_wait) > max_waits:
                    waits = list(si.on_wait)
                    for w in waits[:-max_waits]:
                        ev = mybir.InstEventSemaphore(
                            name=nc.get_next_instruction_name(), ins=[], outs=[]
                        )
                        ev.engine = inst.engine
                        ev.sync_info = mybir.SyncInfo(on_wait=[w], on_update=[])
                        out.append(ev)
                    si.on_wait = waits[-max_waits:]
                out.append(inst)
            insts[:] = out


def build_program():
    nc = bass.Bass()
    xq_d = nc.declare_dram_parameter(
        "xq", [P, KC, B], mybir.dt.float8e4, isOutput=False
    )
    out_d = nc.declare_dram_parameter(
        "top8", [MT, P, 8], mybir.dt.float32, isOutput=True
    )

    with PatchedTileContext(nc) as tc:
        with (
            tc.tile_pool(name="xq_pool", bufs=NCORES) as xq_pool,
            tc.tile_pool(name="st_pool", bufs=ST_BUFS) as st_pool,
            tc.tile_pool(name="cp_pool", bufs=CP_BUFS) as cp_pool,
            tc.tile_pool(name="stg_pool", bufs=3) as stg_pool,
            tc.tile_pool(name="acc_pool", bufs=1) as acc_pool,
            tc.tile_pool(name="psum", bufs=2, space=bass.MemorySpace.PSUM) as psum_pool,
        ):
            # resident fp8 [128, KC, B]; one tile per 1024-column slab so
            # matmuls only depend on the slab they read
            xq_sb = [
                xq_pool.tile([P, KC, 1024], mybir.dt.float8e4, name="xq_rez")
                for _ in range(NCORES)
            ]
            # single queue in strict column order so slab k lands at ~2.9k us
            # (two queues interleave on the DMA bus and scramble arrival);
            # slabs 0-1 go in halves so the first fills start sooner
            for s in range(2):
                for h in range(2):
                    nc.sync.dma_start(
                        xq_sb[s][:, :, ds(h * 512, 512)],
                        xq_d[:, :, ds(s * 1024 + h * 512, 512)],
                    )
            for s in range(2, NCORES):
                nc.sync.dma_start(xq_sb[s][:], xq_d[:, :, ds(s * 1024, 1024)])

            # warm up the PE HAM clock gate during the DMA prologue so the
            # real matmuls run at full clock from the start
            warm_sb = acc_pool.tile([P, 512], mybir.dt.float8e4)
            nc.gpsimd.memset(warm_sb[:], 0.0)
            warm_ps = psum_pool.tile([P, 4, 512], mybir.dt.float32, name="psum")
            for i in range(12):
                nc.tensor.matmul(warm_ps[:, i % 4], warm_sb[:, :P], warm_sb[:])

            l2 = acc_pool.tile([P, 4, 512], mybir.dt.bfloat16)
            l3 = acc_pool.tile([P, 2, 512], mybir.dt.bfloat16)
            l4 = acc_pool.tile([P, 512], mybir.dt.bfloat16)
            out_sb = acc_pool.tile([P, MT, 8], mybir.dt.float32)

            def rhs_ap(kp, col0, width):
                """[128, 2, width] fp8 slice covering k-chunks 2kp,2kp+1."""
                s = col0 // 1024
                o = col0 % 1024
                return xq_sb[s][:, ds(2 * kp, 2), ds(o, width)]

            def fill(pst, m, q):
                """4 DoubleRow accumulation chains -> quarter-row [128,4,512]."""
                for j in range(4):
                    col0 = q * 2048 + j * 512
                    for kp in range(KP):
                        nc.tensor.matmul(
                            pst[:, j],
                            rhs_ap(kp, m * P, P),
                            rhs_ap(kp, col0, 512),
                            start=(kp == 0),
                            stop=(kp == KP - 1),
                            perf_mode=mybir.MatmulPerfMode.DoubleRow,
                        )

            sts = {}
            cps = {}
            stages = {}

            def do_cp(m, q, ps):
                c = cp_pool.tile([P, 4, 512], mybir.dt.bfloat16, name="cp")
                nc.scalar.copy(c[:], ps[:])
                cps[(m, q)] = c

            def st_of(m):
                if m not in sts:
                    sts[m] = st_pool.tile([P, 4, 512], mybir.dt.bfloat16, name="st")
                return sts[m]

            def merge_cp(m, q):
                # fold a banked copy into the rt's running max (bf16 2x mode);
                # first merge pairs the first two copies
                st = st_of(m)
                a = cps.pop((m, q))
                b = cps.pop((m, q - 1), None)
                if b is not None:
                    nc.vector.tensor_tensor(st[:], a[:], b[:], AluOpType.max)
                else:
                    nc.vector.tensor_tensor(st[:], a[:], st[:], AluOpType.max)

            def do_gulp(m, ps, against_cp=None, split=False):
                # one PSUM operand per DVE instruction; the second operand is
                # either a banked copy (st not started) or the running max
                st = st_of(m)
                if split:
                    # consume in 2-bank halves so the tail chain after the
                    # final matmul is half as long
                    for h in range(2):
                        nc.vector.tensor_tensor(
                            st[:, ds(2 * h, 2)],
                            ps[:, ds(2 * h, 2)],
                            st[:, ds(2 * h, 2)],
                            AluOpType.max,
                        )
                    return
                other = cps.pop((m, against_cp))[:] if against_cp is not None else st[:]
                nc.vector.tensor_tensor(st[:], ps[:], other, AluOpType.max)

            def cascade(m):
                if m in stages:
                    # M-plan: merge the four exact per-quarter top-8 tables
                    stg = stages.pop(m)
                    nc.vector.max(out_sb[:, m], stg[:].rearrange("p a b -> p (a b)"))
                    nc.sync.dma_start(out_d[m], out_sb[:, m])
                    return
                st = sts.pop(m)
                nc.vector.tensor_tensor(
                    l3[:], st[:, ds(0, 2)], st[:, ds(2, 2)], AluOpType.max
                )
                nc.vector.tensor_tensor(l4[:], l3[:, 0], l3[:, 1], AluOpType.max)
                nc.vector.max(out_sb[:, m], l4[:])
                nc.sync.dma_start(out_d[m], out_sb[:, m])

            # Per-rt consumer plans (DVE/ACT us per rt):
            #  A: gulp q1 against cp0, merge cp2/cp3 later (DVE 6.7, ACT 6.3)
            #  B: copies first, single gulp at q3 vs running max (6.7, 6.3)
            #  F: all four quarters copied, three bf16 merges (5.5, 8.4)
            def consume(m, q, ps, plan):
                if plan == "A":
                    if q == 0:
                        do_cp(m, q, ps)
                    elif q == 1:
                        do_gulp(m, ps, against_cp=0)
                    else:
                        do_cp(m, q, ps)
                        merge_cp(m, q)
                elif plan in ("B", "S"):
                    if q < 2:
                        do_cp(m, q, ps)
                        if q == 1:
                            merge_cp(m, q)
                    elif q == 2:
                        do_cp(m, q, ps)
                        merge_cp(m, q)
                    else:
                        do_gulp(m, ps, split=(plan == "S" or m == MT - 1))
                elif plan == "C":
                    # ACT-light chain: one copy, then in-place PSUM gulps
                    if q == 0:
                        do_cp(m, q, ps)
                    elif q == 1:
                        do_gulp(m, ps, against_cp=0)
                    else:
                        do_gulp(m, ps, split=(q == 3 and m == MT - 1))
                elif plan == "M":
                    # no ACT at all: exact per-quarter top-8 straight from
                    # PSUM on the DVE (soaks up its idle head window)
                    if m not in stages:
                        stages[m] = stg_pool.tile([P, 4, 8], mybir.dt.float32, name="stg")
                    nc.vector.max(stages[m][:, q], ps[:].rearrange("p a b -> p (a b)"))
                elif plan == "G":
                    # ACT-light endgame plan: only 2 copies; the last quarter
                    # is consumed by split gulps so the tail chain is short
                    if q == 0:
                        do_cp(m, q, ps)
                    elif q == 1:
                        do_gulp(m, ps, against_cp=0)
                    elif q == 2:
                        do_cp(m, q, ps)
                        merge_cp(m, q)
                    else:
                        do_gulp(m, ps, split=True)
                else:  # F
                    do_cp(m, q, ps)
                    if q >= 1:
                        merge_cp(m, q)
                if q == 3 and m not in DEFER_CASC:
                    cascade(m)

            # rts 0-3 column-round-major so the working set tracks DMA slab
            # arrival: round q touches only slabs 2q, 2q+1
            PLAN1 = {m: PLAN1_STR[m] for m in range(4)}
            for q in range(4):
                for m in range(4):
                    ps = psum_pool.tile([P, 4, 512], mybir.dt.float32, name="psum")
                    fill(ps, m, q)
                    consume(m, q, ps, PLAN1[m])

            # rts 4-7 row-major (all slabs resident by now); alternate the
            # ACT-heavy plan F with plan B to balance ACT and DVE
            PLAN2 = {m: PLAN2_STR[m - 4] for m in range(4, MT)}
            for m in range(4, MT):
                for q in range(4):
                    ps = psum_pool.tile([P, 4, 512], mybir.dt.float32, name="psum")
                    fill(ps, m, q)
                    consume(m, q, ps, PLAN2[m])
                    # deferred cascades run while the next rt's fills stream,
                    # keeping the final rt's tail chain unqueued
                    if m - 1 in DEFER_CASC and q == DEFER_Q:
                        cascade(m - 1)

    _split_excess_waits(nc)
    return nc


_nc_cache = None


def kernel(x: np.ndarray) -> np.ndarray:
    global _nc_cache
    assert x.shape == (B, D)

    # --- host: normalize (fp64), scale, quantize, transpose, rotate ---
    x64 = x.astype(np.float64)
    norm = np.sqrt(np.sum(x64 * x64, axis=1, keepdims=True))
    xn = x64 / np.maximum(norm, EPS)
    xq = (xn.T * SCALE).astype(ml_dtypes.float8_e4m3)  # [D, B]
    # [D, B] -> [KC, 128, B] -> [128, KC, B]
    xq = np.ascontiguousarray(xq.reshape(KC, P, B).transpose(1, 0, 2))

    in_maps = []
    for c in range(NCORES):
        r = c * (B // NCORES)
        rolled = np.concatenate((xq[:, :, r:], xq[:, :, :r]), axis=2)
        in_maps.append({"xq": np.ascontiguousarray(rolled)})

    if _nc_cache is None:
        _nc_cache = build_program()
    res = run_bass_kernel_spmd(_nc_cache, in_maps, list(range(NCORES)))

    # --- host: reduce top-8 tables to the scalar loss (fp64) ---
    # top8[c][mt, p, v] -> row c*1024 + mt*128 + p (rotation leaves each
    # core's own rows in place, so the row mapping matches the baseline)
    tops = np.stack([res.results[c]["top8"] for c in range(NCORES)])
    v = tops.reshape(B, 8).astype(np.float64) / (SCALE * SCALE)
    # rank 0 is the self-dot (~1.0); ranks 1..TOPK are the nearest neighbors
    vk = v[:, 1 : 1 + TOPK]  # [B, TOPK]
    d2 = np.maximum(2.0 - 2.0 * vk, 0.0)
    distances = np.sqrt(d2).reshape(-1)
    losses = -np.log(distances + EPS)
    alpha = max(GATE_ALPHA, 1e-6)
    gate = 1.0 / (1.0 + np.exp(-(losses - GATE_THRESHOLD) / alpha))
    lg = losses * gate
    weighted_mean = lg.mean()
    gated_mean = lg.sum() / max(gate.sum(), 1.0)
    out = 0.5 * weighted_mean + 0.5 * gated_mean
    return np.array(out, dtype=np.float32)


# revision 34
# speedup vs baseline: 1.0172x; 1.0094x over previous
"""KoLeo loss (distributed) on 8 Trainium2 NeuronCores.

Strategy: data-parallel over rows, fp8 DoubleRow GEMM. Host normalizes x
(fp64), scales by 16, quantizes to fp8e4 (e4m3), transposes to [D, B] and
stages it per-core ROTATED so each core's own 1024 rows sit at columns
0..1024 — the same program slice is the stationary operand on every core
(no separate lhsT input). Each core computes its [1024, 8192] slice of the
scaled Gram matrix with DoubleRow fp8 matmuls (2 k-chunks per instruction,
0.5 PE cycles/row = 4x bf16), streaming quarter-row [128, 4, 512] tiles
through the 8 PSUM banks.

Top-8 extraction works around two hardware limits (DVE allows only ONE
PSUM operand per instruction; GpSimd cannot touch PSUM or run
tensor ops in this walrus build): the ACT engine drains most PSUM
quarters to SBUF bf16 copies (scalar.copy), and the DVE folds those
copies into a per-row-tile running max with 2x-mode tensor_tensor(max)
— occasionally consuming a quarter directly from PSUM paired against a
banked copy ("gulp") where that balances the engines. Two more fold
levels and the DVE max8 instruction reduce each row-tile to its top-8
slot maxima. Row-tiles 0-3 run column-round-major so the working set
tracks DMA slab arrival (single queue, strict column order); rows 4-7
run row-major. Because rows are unit-norm, the self-dot (=256 scaled)
always ranks first; nearest-neighbor distances follow from
d^2 = 2 - 2*dot. Host reduces the 8x[8,128,8] top-8 tables to the
scalar loss in float64. Fold-slot collisions (two of a row's top-2
neighbors, or self and a neighbor, landing in the same max-slot)
affect ~0.1% of rows and perturb the loss by ~1e-4 relative; fp8 input
quantization contributes ~2e-3 — both far under the 2e-2 gate.
Engine budget per core (TimelineSim): PE 60us, ACT 57us, DVE 42us,
DMA 24us -> 80.7us total vs 233.7us for the bf16 max8-only baseline.
"""

import sys

sys.path.insert(0, "/opt/trn_rl_repo")

import numpy as np
import ml_dtypes

import concourse.bass as bass
import concourse.tile as tile
from concourse import mybir
from concourse.alu_op_type import AluOpType
from concourse.bass import ds, ts
from concourse.vector_clock import ScopedClock
from concourse.bass_utils import run_bass_kernel_spmd

B = 8192
D = 1024
NCORES = 8
P = 128
MT = (B // NCORES) // P  # 8 row-tiles per core
KC = D // P  # 8 k-chunks of 128
KP = KC // 2  # 4 DoubleRow steps (256-contraction each)
NG = 4  # psum groups per row-tile (each 4 banks = 2048 cols)
GW = 2048  # columns per group
SCALE = 16.0

ST_BUFS = 6
CP_BUFS = 28
WARM_N = 8
SPLIT_SLABS = 8
DEFER_CASC = ()
DEFER_Q = 1
PLAN1_STR = "FFFF"
PLAN2_STR = "FFFG"

TOPK = 2
GATE_THRESHOLD = 0.5
GATE_ALPHA = 0.1
EPS = 1e-8


class PatchedTileContext(tile.TileContext):
    """The tail drain in this walrus build only tolerates a single sem wait
    per instruction; spill the rest onto standalone wait instructions."""

    def _drain_and_barrier(self, tick_clock, wait_clock):
        nc = self.nc
        drain_inst = nc.sync.drain()
        wait_clock.add_sem_waits(
            drain_inst.ins, ScopedClock({None: tick_clock.global_clock})
        )
        si = drain_inst.ins.sync_info
        if si is not None and len(si.on_wait) > 1:
            waits = list(si.on_wait)
            si.on_wait = waits[:1]
            id2sem = {h.num: h for h in self.sems.allocated().values()}
            for w in waits[1:]:
                nc.sync.wait_ge(id2sem[w.id], w.wait_value)
        nc.all_engine_barrier()
        popped = nc._tile_sem_poison_stack.pop()
        assert popped is self._sem_poison
        nc.clear_and_free_semaphores(list(self.sems.allocated().values()))
        nc.all_engine_barrier()


def _split_excess_waits(nc, max_waits=1):
    """This walrus build rejects instructions carrying more than one sem
    wait; hoist extras onto standalone EventSemaphore instructions placed
    immediately before the over-subscribed instruction on the same engine
    (engines dispatch in order, so this is semantically identical)."""
    for fn in nc.m.functions:
        for bb in fn.blocks:
            insts = bb.instructions
            out = []
            for inst in insts:
                si = inst.sync_info
                if si is not None and len(si.on_wait) > max_waits:
                    waits = list(si.on_wait)
                    for w in waits[:-max_waits]:
                        ev = mybir.InstEventSemaphore(
                            name=nc.get_next_instruction_name(), ins=[], outs=[]
                        )
                        ev.engine = inst.engine
                        ev.sync_info = mybir.SyncInfo(on_wait=[w], on_update=[])
                        out.append(ev)
                    si.on_wait = waits[-max_waits:]
                out.append(inst)
            insts[:] = out


def build_program():
    nc = bass.Bass()
    xq_d = nc.declare_dram_parameter(
        "xq", [P, KC, B], mybir.dt.float8e4, isOutput=False
    )
    out_d = nc.declare_dram_parameter(
        "top8", [MT, P, 8], mybir.dt.float32, isOutput=True
    )

    with PatchedTileContext(nc) as tc:
        with (
            tc.tile_pool(name="xq_pool", bufs=NCORES) as xq_pool,
            tc.tile_pool(name="st_pool", bufs=ST_BUFS) as st_pool,
            tc.tile_pool(name="cp_pool", bufs=CP_BUFS) as cp_pool,
            tc.tile_pool(name="stg_pool", bufs=3) as stg_pool,
            tc.tile_pool(name="acc_pool", bufs=1) as acc_pool,
            tc.tile_pool(name="psum", bufs=2, space=bass.MemorySpace.PSUM) as psum_pool,
        ):
            # resident fp8 [128, KC, B]; one tile per 1024-column slab so
            # matmuls only depend on the slab they read
            xq_sb = [
                xq_pool.tile([P, KC, 1024], mybir.dt.float8e4, name="xq_rez")
                for _ in range(NCORES)
            ]
            # single queue in strict column order so slab k lands at ~2.9k us
            # (two queues interleave on the DMA bus and scramble arrival);
            # slabs 0-1 go in halves so the first fills start sooner
            for s in range(SPLIT_SLABS):
                for h in range(2):
                    nc.sync.dma_start(
                        xq_sb[s][:, :, ds(h * 512, 512)],
                        xq_d[:, :, ds(s * 1024 + h * 512, 512)],
                    )
            for s in range(SPLIT_SLABS, NCORES):
                nc.sync.dma_start(xq_sb[s][:], xq_d[:, :, ds(s * 1024, 1024)])

            # warm up the PE HAM clock gate during the DMA prologue so the
            # real matmuls run at full clock from the start
            warm_sb = acc_pool.tile([P, 512], mybir.dt.float8e4)
            nc.gpsimd.memset(warm_sb[:], 0.0)
            warm_ps = psum_pool.tile([P, 4, 512], mybir.dt.float32, name="psum")
            for i in range(WARM_N):
                nc.tensor.matmul(warm_ps[:, i % 4], warm_sb[:, :P], warm_sb[:])

            l2 = acc_pool.tile([P, 4, 512], mybir.dt.bfloat16)
            l3 = acc_pool.tile([P, 2, 512], mybir.dt.bfloat16)
            l4 = acc_pool.tile([P, 512], mybir.dt.bfloat16)
            out_sb = acc_pool.tile([P, MT, 8], mybir.dt.float32)

            def rhs_ap(kp, col0, width):
                """[128, 2, width] fp8 slice covering k-chunks 2kp,2kp+1."""
                s = col0 // 1024
                o = col0 % 1024
                return xq_sb[s][:, ds(2 * kp, 2), ds(o, width)]

            def fill(pst, m, q):
                """4 DoubleRow accumulation chains -> quarter-row [128,4,512]."""
                for j in range(4):
                    col0 = q * 2048 + j * 512
                    for kp in range(KP):
                        nc.tensor.matmul(
                            pst[:, j],
                            rhs_ap(kp, m * P, P),
                            rhs_ap(kp, col0, 512),
                            start=(kp == 0),
                            stop=(kp == KP - 1),
                            perf_mode=mybir.MatmulPerfMode.DoubleRow,
                        )

            sts = {}
            cps = {}
            stages = {}

            def do_cp(m, q, ps):
                c = cp_pool.tile([P, 4, 512], mybir.dt.bfloat16, name="cp")
                nc.scalar.copy(c[:], ps[:])
                cps[(m, q)] = c

            def st_of(m):
                if m not in sts:
                    sts[m] = st_pool.tile([P, 4, 512], mybir.dt.bfloat16, name="st")
                return sts[m]

            def merge_cp(m, q):
                # fold a banked copy into the rt's running max (bf16 2x mode);
                # first merge pairs the first two copies
                st = st_of(m)
                a = cps.pop((m, q))
                b = cps.pop((m, q - 1), None)
                if b is not None:
                    nc.vector.tensor_tensor(st[:], a[:], b[:], AluOpType.max)
                else:
                    nc.vector.tensor_tensor(st[:], a[:], st[:], AluOpType.max)

            def do_gulp(m, ps, against_cp=None, split=False):
                # one PSUM operand per DVE instruction; the second operand is
                # either a banked copy (st not started) or the running max
                st = st_of(m)
                if split:
                    # consume in 2-bank halves so the tail chain after the
                    # final matmul is half as long
                    for h in range(2):
                        nc.vector.tensor_tensor(
                            st[:, ds(2 * h, 2)],
                            ps[:, ds(2 * h, 2)],
                            st[:, ds(2 * h, 2)],
                            AluOpType.max,
                        )
                    return
                other = cps.pop((m, against_cp))[:] if against_cp is not None else st[:]
                nc.vector.tensor_tensor(st[:], ps[:], other, AluOpType.max)

            def cascade(m):
                if m in stages:
                    # M/N-plan: merge the exact per-(half)quarter top-8 tables
                    stg = stages.pop(m)
                    nc.vector.max(out_sb[:, m], stg[:].rearrange("p a b -> p (a b)"))
                    nc.sync.dma_start(out_d[m], out_sb[:, m])
                    return
                st = sts.pop(m)
                nc.vector.tensor_tensor(
                    l3[:], st[:, ds(0, 2)], st[:, ds(2, 2)], AluOpType.max
                )
                nc.vector.tensor_tensor(l4[:], l3[:, 0], l3[:, 1], AluOpType.max)
                nc.vector.max(out_sb[:, m], l4[:])
                nc.sync.dma_start(out_d[m], out_sb[:, m])

            # Per-rt consumer plans (DVE/ACT us per rt):
            #  A: gulp q1 against cp0, merge cp2/cp3 later (DVE 6.7, ACT 6.3)
            #  B: copies first, single gulp at q3 vs running max (6.7, 6.3)
            #  F: all four quarters copied, three bf16 merges (5.5, 8.4)
            def consume(m, q, ps, plan):
                if plan == "A":
                    if q == 0:
                        do_cp(m, q, ps)
                    elif q == 1:
                        do_gulp(m, ps, against_cp=0)
                    else:
                        do_cp(m, q, ps)
                        merge_cp(m, q)
                elif plan in ("B", "S"):
                    if q < 2:
                        do_cp(m, q, ps)
                        if q == 1:
                            merge_cp(m, q)
                    elif q == 2:
                        do_cp(m, q, ps)
                        merge_cp(m, q)
                    else:
                        do_gulp(m, ps, split=(plan == "S" or m == MT - 1))
                elif plan == "C":
                    # ACT-light chain: one copy, then in-place PSUM gulps
                    if q == 0:
                        do_cp(m, q, ps)
                    elif q == 1:
                        do_gulp(m, ps, against_cp=0)
                    else:
                        do_gulp(m, ps, split=(q == 3 and m == MT - 1))
                elif plan == "N":
                    # like M but two half-quarter max8s: psum banks release
                    # in halves, keeping the ring fed
                    if m not in stages:
                        stages[m] = stg_pool.tile([P, 8, 8], mybir.dt.float32, name="stg")
                    for h in range(2):
                        nc.vector.max(
                            stages[m][:, 2 * q + h],
                            ps[:, ds(2 * h, 2)].rearrange("p a b -> p (a b)"),
                        )
                elif plan == "M":
                    # no ACT at all: exact per-quarter top-8 straight from
                    # PSUM on the DVE (soaks up its idle head window)
                    if m not in stages:
                        stages[m] = stg_pool.tile([P, 4, 8], mybir.dt.float32, name="stg")
                    nc.vector.max(stages[m][:, q], ps[:].rearrange("p a b -> p (a b)"))
                elif plan == "G":
                    # ACT-light endgame plan: only 2 copies; the last quarter
                    # is consumed by split gulps so the tail chain is short
                    if q == 0:
                        do_cp(m, q, ps)
                    elif q == 1:
                        do_gulp(m, ps, against_cp=0)
                    elif q == 2:
                        do_cp(m, q, ps)
                        merge_cp(m, q)
                    else:
                        do_gulp(m, ps, split=True)
                else:  # F
                    do_cp(m, q, ps)
                    if q >= 1:
                        merge_cp(m, q)
                if q == 3 and m not in DEFER_CASC:
                    cascade(m)

            # rts 0-3 column-round-major so the working set tracks DMA slab
            # arrival: round q touches only slabs 2q, 2q+1
            PLAN1 = {m: PLAN1_STR[m] for m in range(4)}
            for q in range(4):
                for m in range(4):
                    ps = psum_pool.tile([P, 4, 512], mybir.dt.float32, name="psum")
                    fill(ps, m, q)
                    consume(m, q, ps, PLAN1[m])

            # rts 4-7 row-major (all slabs resident by now); alternate the
            # ACT-heavy plan F with plan B to balance ACT and DVE
            PLAN2 = {m: PLAN2_STR[m - 4] for m in range(4, MT)}
            for m in range(4, MT):
                for q in range(4):
                    ps = psum_pool.tile([P, 4, 512], mybir.dt.float32, name="psum")
                    fill(ps, m, q)
                    consume(m, q, ps, PLAN2[m])
                    # deferred cascades run while the next rt's fills stream,
                    # keeping the final rt's tail chain unqueued
                    if m - 1 in DEFER_CASC and q == DEFER_Q:
                        cascade(m - 1)

    _split_excess_waits(nc)
    return nc


_nc_cache = None


def kernel(x: np.ndarray) -> np.ndarray:
    global _nc_cache
    assert x.shape == (B, D)

    # --- host: normalize (fp64), scale, quantize, transpose, rotate ---
    x64 = x.astype(np.float64)
    norm = np.sqrt(np.sum(x64 * x64, axis=1, keepdims=True))
    xn = x64 / np.maximum(norm, EPS)
    xq = (xn.T * SCALE).astype(ml_dtypes.float8_e4m3)  # [D, B]
    # [D, B] -> [KC, 128, B] -> [128, KC, B]
    xq = np.ascontiguousarray(xq.reshape(KC, P, B).transpose(1, 0, 2))

    in_maps = []
    for c in range(NCORES):
        r = c * (B // NCORES)
        rolled = np.concatenate((xq[:, :, r:], xq[:, :, :r]), axis=2)
        in_maps.append({"xq": np.ascontiguousarray(rolled)})

    if _nc_cache is None:
        _nc_cache = build_program()
    res = run_bass_kernel_spmd(_nc_cache, in_maps, list(range(NCORES)))

    # --- host: reduce top-8 tables to the scalar loss (fp64) ---
    # top8[c][mt, p, v] -> row c*1024 + mt*128 + p (rotation leaves each
    # core's own rows in place, so the row mapping matches the baseline)
    tops = np.stack([res.results[c]["top8"] for c in range(NCORES)])
    v = tops.reshape(B, 8).astype(np.float64) / (SCALE * SCALE)
    # rank 0 is the self-dot (~1.0); ranks 1..TOPK are the nearest neighbors
    vk = v[:, 1 : 1 + TOPK]  # [B, TOPK]
    d2 = np.maximum(2.0 - 2.0 * vk, 0.0)
    distances = np.sqrt(d2).reshape(-1)
    losses = -np.log(distances + EPS)
    alpha = max(GATE_ALPHA, 1e-6)
    gate = 1.0 / (1.0 + np.exp(-(losses - GATE_THRESHOLD) / alpha))
    lg = losses * gate
    weighted_mean = lg.mean()
    gated_mean = lg.sum() / max(gate.sum(), 1.0)
    out = 0.5 * weighted_mean + 0.5 * gated_mean
    return np.array(out, dtype=np.float32)
